# revision 1
# baseline (speedup 1.0000x reference)
"""Causal self-attention (B=8, T=1024, C=768, H=12) on 8 Trainium2 NeuronCores.

Sharding: data parallel — one batch element per core, no collectives.

Per-core Bass/Tile kernel (all matmuls in float32r: 1 cyc/row at N>=256,
~4x fp32 throughput, ~2.5e-4 scale-relative rounding):
  x'^T = [x^T; ones]                       (PE-transposed, bias-via-ones trick)
  Q^T, K^T = Wqkv'.T @ x'^T                (features on partitions)
  V = x'^T.T @ Wv'                         (natural layout, per-head + ones col)
  per head: S^T = K_h^T.T @ Q_h^T          (k on partitions, exact causal chunks)
    causal mask on the diagonal 128x128 block added on PSUM via a bf16
    matmul (negmask^T.T @ I), then ACT exp(s/8) -> ragged P^T
    O'^T = [V_h | 1].T @ P^T               (row 64 = softmax denominator)
    normalize: DVE reciprocal + tiny DMA to partition 0 + GPSIMD
    partition_broadcast + DVE multiply (odd heads take an SBUF->SBUF DMA to
    reach partitions 64..127 — DVE lanes are partition-locked)
  y = attn'^T.T @ Wp' + bias
"""
import sys
from contextlib import ExitStack

import numpy as np

for _p in ("/opt/trn_rl_repo", "/root/.axon_site/_ro/trn_rl_repo"):
    if _p not in sys.path:
        sys.path.insert(0, _p)

import concourse.bass as bass  # noqa: E402
import concourse.mybir as mybir  # noqa: E402

F32 = mybir.dt.float32
BF16 = mybir.dt.bfloat16
R32 = mybir.dt.float32r
AF = mybir.ActivationFunctionType
OP = mybir.AluOpType

B, T, C, H, D = 8, 1024, 768, 12, 64
N_CORES = 8


def _chunks_512(a, b):
    out = []
    while a < b:
        nxt = min((a // 512 + 1) * 512, b)
        out.append((a, nxt))
        a = nxt
    return out


def _emit_attention(tc, io, use_pbcast=True):
    nc = tc.nc
    NT = T // 128
    NC = C // 128

    def r(ap):
        return ap.bitcast(R32)

    with ExitStack() as stack:
        persist = stack.enter_context(tc.tile_pool(name="persist", bufs=1))
        consts = stack.enter_context(tc.tile_pool(name="consts", bufs=1))
        negmT = consts.tile([128, 128], BF16, tag="negmT")
        idb = consts.tile([128, 128], BF16, tag="idb")
        nc.sync.dma_start(negmT[:], io["negmaskT16"])
        nc.sync.dma_start(idb[:], io["identity16"])
        bqt_sb = consts.tile([128, 3 * C // 128], F32, tag="bqt")
        nc.sync.dma_start(bqt_sb[:], io["bqkvT"])
        bb_sb = consts.tile([128, 2 * C], F32, tag="bb")
        nc.sync.dma_start(bb_sb[:], io["bias_bcast"])
        ones_sb = consts.tile([128, max(T, 128)], F32, tag="ones_sb")
        nc.sync.dma_start(ones_sb[:], io["ones"][:, 0:max(T, 128)])
        ones_l = ones_sb

        qt = persist.tile([128, NC, T], F32, tag="qt")
        kt_ = persist.tile([128, NC, T], F32, tag="kt")
        vp = persist.tile([128, NT, H, D + 1], F32, tag="vp")

        # ---------------- phase 1: x^T, Q^T, K^T, V ----------------
        with tc.tile_pool(name="p1w", bufs=1) as p1w, \
             tc.tile_pool(name="ps1b", bufs=6, space="PSUM") as ps1b:
            x1t = p1w.tile([128, NC + 1, T], F32, tag="x1t")
            wq_sb = p1w.tile([128, NC + 1, 3 * C], F32, tag="wq")
            nc.vector.tensor_copy(r(x1t[0:1, NC, :]), ones_sb[0:1, 0:T])
            nc.vector.tensor_copy(
                r(vp[:, :, :, 64]),
                ones_sb[:, 0:NT * H].rearrange("p (t h) -> p t h", h=H))

            # x arrives pre-transposed from host staging: straight DMAs,
            # interleaved with the column-sliced weight groups
            wgroups = _chunks_512(0, 3 * C)
            for c in range(NC):
                nc.sync.dma_start(r(x1t[:, c, :]),
                                  r(io["xT"][c * 128:(c + 1) * 128, :]))
                for g in range(len(wgroups)):
                    if g % NC == c:
                        w0, w1 = wgroups[g]
                        for kt in range(NC):
                            nc.sync.dma_start(
                                r(wq_sb[:, kt, w0:w1]),
                                r(io["wqkv"][kt * 128:(kt + 1) * 128, w0:w1]))
                        nc.sync.dma_start(r(wq_sb[0:1, NC, w0:w1]),
                                          r(io["bqkv"][None, w0:w1]))

            m_order = [ft + o for ft in range(NC) for o in (0, NC)]
            for m in m_order:
                dest = qt if m < NC else kt_
                mm = m % NC
                for (a, b) in _chunks_512(0, T):
                    ps = ps1b.tile([128, b - a], F32, tag="ps_mm")
                    for kt in range(NC):
                        nc.tensor.matmul(
                            ps[:], r(wq_sb[:, kt, m * 128:(m + 1) * 128]),
                            r(x1t[:, kt, a:b]), start=(kt == 0),
                            stop=(kt == NC - 1))
                    # bias is per-partition here: fuse it into the copy
                    nc.vector.tensor_scalar_add(r(dest[:, mm, a:b]), ps[:],
                                                bqt_sb[:, m:m + 1])

            for t in range(NT):
                for (n0, n1) in _chunks_512(0, C):
                    ps = ps1b.tile([128, n1 - n0], F32, tag="ps_mm")
                    for kt in range(NC):
                        nc.tensor.matmul(
                            ps[:], r(x1t[:, kt, t * 128:(t + 1) * 128]),
                            r(wq_sb[:, kt, 2 * C + n0:2 * C + n1]),
                            start=(kt == 0), stop=(kt == NC - 1))
                    h0, h1 = n0 // D, n1 // D
                    nc.vector.tensor_tensor(
                        r(vp[:, t, h0:h1, 0:D]),
                        ps[:].rearrange("p (h d) -> p h d", d=D),
                        bb_sb[:, n0:n1].rearrange("p (h d) -> p h d", d=D),
                        OP.add)

        # ---------------- phase 2: attention ----------------
        off = [0] * (NT + 1)
        for i in range(NT):
            off[i + 1] = off[i] + (T - 128 * i)
        PTW = off[NT]

        p23 = stack.enter_context(tc.tile_pool(name="p23", bufs=1))
        attnT = p23.tile([128, NC + 1, T], F32, tag="attnT")
        wpp = p23.tile([128, NC + 1, C], F32, tag="wpp")
        for kt in range(NC):
            nc.sync.dma_start(r(wpp[:, kt, :]),
                              r(io["wp"][kt * 128:(kt + 1) * 128, :]))
        nc.sync.dma_start(r(wpp[0:1, NC, :]), r(io["bp"][None, :]))
        nc.vector.tensor_copy(r(attnT[0:1, NC, :]), ones_sb[0:1, 0:T])

        with tc.tile_pool(name="p2", bufs=2) as p2, \
             tc.tile_pool(name="p2o", bufs=3) as p2o, \
             tc.tile_pool(name="ps2a", bufs=2, space="PSUM") as ps2a, \
             tc.tile_pool(name="ps2b", bufs=3, space="PSUM") as ps2b, \
             tc.tile_pool(name="ps2c", bufs=1, space="PSUM") as ps2c:
            head_order = [hp * 2 + o for hp in range(H // 2) for o in (1, 0)]
            for h in head_order:
                p0 = 64 * (h % 2)      # partition base of this head's features
                ft = h // 2            # feature tile
                pt_sb = p2.tile([128, PTW], F32, tag="pt", name=f"pt{h}")
                for kt in range(NT):
                    base = (kt * 128 // 512) * 512
                    ps_s = ps2a.tile([128, T - base], F32, tag="ps_s")
                    for (a, b) in _chunks_512(kt * 128, T):
                        diag = a == kt * 128
                        nc.tensor.matmul(
                            ps_s[:, a - base:b - base],
                            r(kt_[p0:p0 + 64, ft, kt * 128:(kt + 1) * 128]),
                            r(qt[p0:p0 + 64, ft, a:b]),
                            start=True, stop=not diag)
                        if diag:
                            nc.tensor.matmul(ps_s[:, a - base:a - base + 128],
                                             negmT[:], idb[:],
                                             start=False, stop=True)
                    # one exp per strip -> ragged P^T
                    nc.scalar.activation(
                        r(pt_sb[:, off[kt]:off[kt + 1]]),
                        ps_s[:, kt * 128 - base:T - base],
                        AF.Exp, bias=0.0, scale=1.0 / np.sqrt(D))
                if True:
                    for (q0, q1) in _chunks_512(0, T):
                        kt_max = q1 // 128
                        ps_o = ps2b.tile([65, 512], F32, tag="ps_o")
                        for kt in range(kt_max):
                            a = max(q0, kt * 128)
                            rhs = r(pt_sb[:, off[kt] + a - kt * 128:
                                          off[kt] + q1 - kt * 128])
                            nc.tensor.matmul(
                                ps_o[:, a - q0:q1 - q0],
                                r(vp[:, kt, h, :]), rhs,
                                start=(kt == 0), stop=(kt == kt_max - 1))
                        w = q1 - q0
                        dn = p2o.tile([65, 512], F32, tag="dn")
                        nc.vector.reciprocal(dn[64:65, 0:w], ps_o[64:65, 0:w])
                        bc = p2o.tile([128, 512], F32, tag="bc")
                        if use_pbcast:
                            # partition_broadcast reads partition 0 only:
                            # stage the denominator row there via a tiny DMA
                            dn0 = p2o.tile([1, 512], F32, tag="dn0")
                            nc.sync.dma_start(dn0[0:1, 0:w], dn[64:65, 0:w])
                            nc.gpsimd.partition_broadcast(bc[:, 0:w], dn0[0:1, 0:w])
                        else:
                            ps_bc = ps2c.tile([128, 512], F32, tag="ps_bc")
                            nc.tensor.matmul(ps_bc[:, 0:w], r(ones_l[64:65, 0:128]),
                                             r(dn[64:65, 0:w]), start=True, stop=True)
                            nc.vector.tensor_copy(bc[:, 0:w], ps_bc[:, 0:w])
                        if h % 2 == 0:
                            nc.vector.tensor_tensor(
                                r(attnT[0:64, ft, q0:q1]),
                                ps_o[0:64, 0:w], bc[0:64, 0:w], OP.mult)
                        else:
                            o_n = p2o.tile([64, 512], F32, tag="o_n")
                            nc.vector.tensor_tensor(
                                r(o_n[:, 0:w]), ps_o[0:64, 0:w],
                                bc[0:64, 0:w], OP.mult)
                            nc.sync.dma_start(
                                r(attnT[64:128, ft, q0:q1]), r(o_n[:, 0:w]))

        # ---------------- phase 3: projection ----------------
        with tc.tile_pool(name="p3", bufs=3) as p3, \
             tc.tile_pool(name="ps3", bufs=4, space="PSUM") as ps3:
            for t in range(NT):
                for (n0, n1) in _chunks_512(0, C):
                    ps_y = ps3.tile([128, n1 - n0], F32, tag="ps_y")
                    for kt in range(NC):
                        nc.tensor.matmul(
                            ps_y[:], r(attnT[:, kt, t * 128:(t + 1) * 128]),
                            r(wpp[:, kt, n0:n1]),
                            start=(kt == 0), stop=(kt == NC - 1))
                    y_sb = p3.tile([128, n1 - n0], F32, tag="y_sb")
                    nc.vector.tensor_tensor(y_sb[:], ps_y[:],
                                            bb_sb[:, C + n0:C + n1], OP.add)
                    nc.sync.dma_start(io["y"][t * 128:(t + 1) * 128, n0:n1], y_sb[:])


def build_nc():
    from concourse import bacc
    import concourse.tile as tile
    nc = bacc.Bacc("TRN2", target_bir_lowering=False, debug=False,
                   enable_asserts=True, num_devices=N_CORES)
    io = {
        "xT": nc.dram_tensor("xT", [C, T], F32, kind="ExternalInput").ap(),
        "wqkv": nc.dram_tensor("wqkv", [C, 3 * C], F32, kind="ExternalInput").ap(),
        "bqkv": nc.dram_tensor("bqkv", [3 * C], F32, kind="ExternalInput").ap(),
        "bqkvT": nc.dram_tensor("bqkvT", [128, 3 * C // 128], F32,
                                kind="ExternalInput").ap(),
        "bias_bcast": nc.dram_tensor("bias_bcast", [128, 2 * C], F32,
                                     kind="ExternalInput").ap(),
        "wp": nc.dram_tensor("wp", [C, C], F32, kind="ExternalInput").ap(),
        "bp": nc.dram_tensor("bp", [C], F32, kind="ExternalInput").ap(),
        "ones": nc.dram_tensor("ones", [128, 1024], F32, kind="ExternalInput").ap(),
        "negmaskT16": nc.dram_tensor("negmaskT16", [128, 128], BF16,
                                     kind="ExternalInput").ap(),
        "identity16": nc.dram_tensor("identity16", [128, 128], BF16,
                                     kind="ExternalInput").ap(),
        "y": nc.dram_tensor("y", [T, C], F32, kind="ExternalOutput").ap(),
    }
    with tile.TileContext(nc) as tc:
        _emit_attention(tc, io)
    nc.compile()
    return nc


def host_consts():
    import ml_dtypes
    negmask = np.where(np.triu(np.ones((128, 128), dtype=bool)), 0.0,
                       -1e9).astype(np.float32)
    return {
        "ones": np.ones((128, 1024), dtype=np.float32),
        "negmaskT16": np.ascontiguousarray(negmask.T).astype(ml_dtypes.bfloat16),
        "identity16": np.eye(128, dtype=ml_dtypes.bfloat16),
    }


_NC_CACHE = None


def _get_nc():
    global _NC_CACHE
    if _NC_CACHE is None:
        _NC_CACHE = build_nc()
    return _NC_CACHE


def make_in_maps(x, c_attn_kernel, c_attn_bias, c_proj_kernel, c_proj_bias):
    consts = host_consts()
    wqkv = np.ascontiguousarray(c_attn_kernel, dtype=np.float32)
    bqkv = np.ascontiguousarray(c_attn_bias, dtype=np.float32)
    bqkvT = np.ascontiguousarray(bqkv.reshape(3 * C // 128, 128).T)
    wp = np.ascontiguousarray(c_proj_kernel, dtype=np.float32)
    bp = np.ascontiguousarray(c_proj_bias, dtype=np.float32)
    bias_bcast = np.ascontiguousarray(
        np.tile(np.concatenate([bqkv[2 * C:], bp]), (128, 1)))
    in_maps = []
    for bb in range(N_CORES):
        m = {"xT": np.ascontiguousarray(x[bb].T, dtype=np.float32),
             "wqkv": wqkv, "bqkv": bqkv, "bqkvT": bqkvT, "wp": wp, "bp": bp,
             "bias_bcast": bias_bcast}
        m.update(consts)
        in_maps.append(m)
    return in_maps


def kernel(x, c_attn_kernel, c_attn_bias, c_proj_kernel, c_proj_bias):
    from concourse.bass_utils import run_bass_kernel_spmd
    x = np.asarray(x)
    assert x.shape == (B, T, C), x.shape
    nc = _get_nc()
    in_maps = make_in_maps(x, c_attn_kernel, c_attn_bias, c_proj_kernel,
                           c_proj_bias)
    res = run_bass_kernel_spmd(nc, in_maps, core_ids=list(range(N_CORES)))
    y = np.stack([res.results[bb]["y"] for bb in range(N_CORES)]).astype(np.float32)
    return y



# revision 24
# speedup vs baseline: 4.4953x; 4.4953x over previous
"""Causal self-attention (B=8, T=1024, C=768, H=12) on 8 Trainium2 NeuronCores.

Sharding: data parallel - one batch element per core, no collectives.

v2: fp16 matmul datapath (1 cyc/row at any width, half the DMA bytes of
fp32), with phase 1 (QKV projection) and phase 2 (attention) software-
pipelined per feature-tile so the Activation engine's softmax exp (~60us
total) hides under PE matmul work:

  prologue: QKV(ft=0)
  for ft in 0..5:  weave[ S(2ft), S(2ft+1) | QKV(ft+1) | O(2ft-2), O(2ft-1) ]
  epilogue: O(10), O(11), projection

Per-head attention keeps the baseline's transposed dataflow:
  S^T strip kt = K_h^T.T @ Q_h^T   [keys on psum partitions, ragged causal]
  exp on ACT (scale=1/8) -> P^T fp16; causal diag mask = fp16 0/1 multiply
  on DVE (no PE mask matmuls)
  O'^T = [V_h | 1].T @ P^T          (psum row 64 = softmax denominator)
  normalize: DVE reciprocal + tiny DMA to partition 0 + gpsimd
  partition_broadcast + DVE multiply (odd heads DMA to partitions 64..127)
"""
import sys
from contextlib import ExitStack

import numpy as np

for _p in ("/opt/trn_rl_repo", "/root/.axon_site/_ro/trn_rl_repo"):
    if _p not in sys.path:
        sys.path.insert(0, _p)

import concourse.bass as bass  # noqa: E402
import concourse.mybir as mybir  # noqa: E402

F32 = mybir.dt.float32
F16 = mybir.dt.float16
AF = mybir.ActivationFunctionType
OP = mybir.AluOpType

B, T, C, H, D = 8, 1024, 768, 12, 64
N_CORES = 8
NC = C // 128   # 6 contraction tiles
NT = T // 128   # 8 time tiles
NF = NC         # 6 feature tiles (2 heads each)

# ragged P^T strip offsets: strip kt holds queries kt*128..T
_OFF = [0] * (NT + 1)
for _i in range(NT):
    _OFF[_i + 1] = _OFF[_i] + (T - 128 * _i)
PTW = _OFF[NT]  # 4608


def _chunks_512(a, b):
    out = []
    while a < b:
        nxt = min((a // 512 + 1) * 512, b)
        out.append((a, nxt))
        a = nxt
    return out


def _weave(*lists):
    """Proportionally interleave several lists of closures."""
    items = []
    for li, lst in enumerate(lists):
        n = len(lst)
        for i, u in enumerate(lst):
            items.append(((i + 0.5) / n, li, i, u))
    items.sort(key=lambda x: (x[0], x[1]))
    return [u for _, _, _, u in items]


def _emit_attention(tc, io):
    nc = tc.nc

    # Pools persist across repeat bodies (allocated once, never released):
    # closing pools per body would emit drain barriers that serialize
    # bodies and kill cross-body DMA prefetch.
    pools = getattr(tc, "_v2_pools", None)
    if pools is None:
        pools = {
            "consts": tc.alloc_tile_pool(name="consts", bufs=2),
            "px": tc.alloc_tile_pool(name="px", bufs=1),
            "pw": tc.alloc_tile_pool(name="pw", bufs=NF),
            "pqkv": tc.alloc_tile_pool(name="pqkv", bufs=2),
            "pv": tc.alloc_tile_pool(name="pv", bufs=3),
            "p2": tc.alloc_tile_pool(name="p2", bufs=5),
            "p2o": tc.alloc_tile_pool(name="p2o", bufs=3),
            "py": tc.alloc_tile_pool(name="py", bufs=3),
            "ps1": tc.alloc_tile_pool(name="ps1", bufs=2, space="PSUM"),
            "ps2a": tc.alloc_tile_pool(name="ps2a", bufs=3, space="PSUM"),
            "ps2b": tc.alloc_tile_pool(name="ps2b", bufs=3, space="PSUM"),
        }
        tc._v2_pools = pools
    consts, px, pw = pools["consts"], pools["px"], pools["pw"]
    pqkv, pv, p2 = pools["pqkv"], pools["pv"], pools["p2"]
    p2o, py = pools["p2o"], pools["py"]
    ps1, ps2a, ps2b = pools["ps1"], pools["ps2a"], pools["ps2b"]

    if True:
        bqt_sb = consts.tile([128, 3 * C // 128], F32, tag="bqt")
        bb_sb = consts.tile([128, 2 * C], F32, tag="bb")
        mask_sb = consts.tile([128, 128], F16, tag="mask01")

        x1t = px.tile([128, NC, T], F16, tag="x1t")
        attnT = px.tile([128, NC, T], F16, tag="attnT")
        wpp = px.tile([128, NC, C], F16, tag="wpp")

        # weight tiles per feature block: [kt, {q,k,v} x 128] columns, one
        # DMA each (host pre-shuffles wqkv ft-major). DMA priority order:
        # bqt (needed by the first psum drain), wq(0), x per k-tile on the
        # Activation DGE (QKV(0) starts on tile 0 ASAP), then the rest.
        wq_t = [None] * NF
        for ft in range(NF):
            wq_t[ft] = pw.tile([128, NC, 3 * 128], F16, tag="wq",
                               name=f"wq{ft}")

        def load_wq(ft):
            nc.sync.dma_start(
                wq_t[ft][:],
                io["wqkv"].rearrange("(k p) n -> p k n", p=128)[
                    :, :, ft * 384:(ft + 1) * 384])

        nc.sync.dma_start(bqt_sb[:], io["bqkvT"])
        load_wq(0)
        for kt in range(NC):
            nc.scalar.dma_start(
                x1t[:, kt, :],
                io["xT"].rearrange("(k p) t -> p k t", p=128)[:, kt, :])
        nc.sync.dma_start(bb_sb[:], io["bias_bcast"])
        nc.sync.dma_start(mask_sb[:], io["mask01"])
        for ft in range(1, NF):
            load_wq(ft)
        nc.sync.dma_start(
            wpp[:], io["wp"].rearrange("(k p) n -> p k n", p=128))

        qt_t = [None] * NF
        kt_t = [None] * NF
        vp_t = [None] * NF
        pt_t = [None] * H

        def make_qkv_units(ft):
            """20 closures: Q 2 chunks, K 2 chunks, V 8 t-tiles."""
            units = []

            def alloc():
                qt_t[ft] = pqkv.tile([128, T], F16, tag="qt", name=f"qt{ft}")
                kt_t[ft] = pqkv.tile([128, T], F16, tag="kt", name=f"kt{ft}")
                vp_t[ft] = pv.tile([128, NT, 2, D + 1], F16, tag="vp",
                                   name=f"vp{ft}")
                nc.vector.memset(vp_t[ft][:, :, :, D], 1.0)

            def qk_unit(which, a, b, first):
                def emit():
                    if first:
                        alloc()
                    dest = qt_t[ft] if which == 0 else kt_t[ft]
                    ps = ps1.tile([128, 512], F32, tag="ps_qk")
                    for kt in range(NC):
                        nc.tensor.matmul(
                            ps[:, 0:b - a],
                            wq_t[ft][:, kt, which * 128:(which + 1) * 128],
                            x1t[:, kt, a:b], start=(kt == 0),
                            stop=(kt == NC - 1))
                    nc.vector.tensor_scalar_add(
                        dest[:, a:b], ps[:, 0:b - a],
                        bqt_sb[:, which * NC + ft:which * NC + ft + 1])
                return emit

            def v_unit(t):
                def emit():
                    ps = ps1.tile([128, 512], F32, tag="ps_qk")
                    for kt in range(NC):
                        nc.tensor.matmul(
                            ps[:, 0:128], x1t[:, kt, t * 128:(t + 1) * 128],
                            wq_t[ft][:, kt, 256:384], start=(kt == 0),
                            stop=(kt == NC - 1))
                    nc.vector.tensor_tensor(
                        vp_t[ft][:, t, :, 0:D],
                        ps[:, 0:128].rearrange("p (h d) -> p h d", d=D),
                        bb_sb[:, ft * 128:(ft + 1) * 128].rearrange(
                            "p (h d) -> p h d", d=D),
                        OP.add)
                return emit

            units.append(qk_unit(0, 0, 512, True))
            units.append(qk_unit(0, 512, 1024, False))
            units.append(qk_unit(1, 0, 512, False))
            units.append(qk_unit(1, 512, 1024, False))
            for t in range(NT):
                units.append(v_unit(t))
            return units

        def make_s_units(h):
            """12 closures: one per (strip, 512-chunk)."""
            ft, p0 = h // 2, 64 * (h % 2)
            units = []

            def alloc():
                pt_t[h] = p2.tile([128, PTW], F16, tag="pt", name=f"pt{h}")

            def s_unit(kt, a, b, first):
                def emit():
                    if first:
                        alloc()
                    pt = pt_t[h]
                    ps = ps2a.tile([128, 512], F32, tag="ps_s")
                    nc.tensor.matmul(
                        ps[:, 0:b - a],
                        kt_t[ft][p0:p0 + 64, kt * 128:(kt + 1) * 128],
                        qt_t[ft][p0:p0 + 64, a:b], start=True, stop=True)
                    o = _OFF[kt] + a - kt * 128
                    nc.scalar.activation(
                        pt[:, o:o + b - a], ps[:, 0:b - a],
                        AF.Exp, bias=0.0, scale=1.0 / np.sqrt(D))
                    if a == kt * 128:
                        # causal 0/1 mask on the diagonal 128x128 block
                        nc.vector.tensor_tensor(
                            pt[:, o:o + 128], pt[:, o:o + 128], mask_sb[:],
                            OP.mult)
                return emit

            first = True
            for kt in range(NT):
                for (a, b) in _chunks_512(kt * 128, T):
                    units.append(s_unit(kt, a, b, first))
                    first = False
            return units

        def make_o_units(h):
            """2 closures, one per 512-query chunk. Each emits the matmuls +
            recip + broadcast kickoff, and schedules the DVE multiply a
            couple of units later (so DVE never head-of-line blocks on the
            broadcast chain)."""
            ft, p0 = h // 2, 64 * (h % 2)
            units = []

            def o_unit(q0, q1):
                def emit():
                    pt = pt_t[h]
                    kt_max = q1 // 128
                    ps_o = ps2b.tile([65, 512], F32, tag="ps_o")
                    for kt in range(kt_max):
                        a = max(q0, kt * 128)
                        rhs = pt[:, _OFF[kt] + a - kt * 128:
                                 _OFF[kt] + q1 - kt * 128]
                        nc.tensor.matmul(
                            ps_o[:, a - q0:q1 - q0], vp_t[ft][:, kt, h % 2, :],
                            rhs, start=(kt == 0), stop=(kt == kt_max - 1))
                    w = q1 - q0
                    dn = p2o.tile([65, 512], F32, tag="dn")
                    nc.vector.reciprocal(dn[64:65, 0:w], ps_o[64:65, 0:w])
                    # partition_broadcast reads partition 0 only: stage the
                    # denominator row there via a tiny DMA (gpsimd DGE — it
                    # precedes the broadcast on the same queue anyway)
                    dn0 = p2o.tile([1, 512], F32, tag="dn0")
                    nc.sync.dma_start(dn0[0:1, 0:w], dn[64:65, 0:w])
                    bc = p2o.tile([128, 512], F32, tag="bc")
                    nc.gpsimd.partition_broadcast(bc[:, 0:w], dn0[0:1, 0:w])

                    def finish():
                        if h % 2 == 0:
                            nc.vector.tensor_tensor(
                                attnT[0:64, ft, q0:q1], ps_o[0:64, 0:w],
                                bc[0:64, 0:w], OP.mult)
                        else:
                            # DVE lanes are partition-locked: stage + DMA up
                            o_n = p2o.tile([64, 512], F16, tag="o_n")
                            nc.vector.tensor_tensor(
                                o_n[:, 0:w], ps_o[0:64, 0:w], bc[0:64, 0:w],
                                OP.mult)
                            nc.sync.dma_start(attnT[64:128, ft, q0:q1],
                                              o_n[:, 0:w])
                    return finish
                return emit

            units.append(o_unit(0, 512))
            units.append(o_unit(512, 1024))
            return units

        # ---------------- emission driver ----------------
        # Units may return a "finish" closure, deferred ~2 units so slow
        # dependency chains (normalize broadcast) never block DVE in-order.
        pending = []
        ecount = [0]

        def run_unit(u):
            for item in pending[:]:
                if item[0] <= ecount[0]:
                    item[1]()
                    pending.remove(item)
            r = u()
            ecount[0] += 1
            if callable(r):
                pending.append((ecount[0] + 2, r))

        def make_p3_unit(t, n0, n1):
            def emit():
                ps_y = ps1.tile([128, 512], F32, tag="ps_qk")
                for kt in range(NC):
                    nc.tensor.matmul(
                        ps_y[:, 0:n1 - n0], attnT[:, kt, t * 128:(t + 1) * 128],
                        wpp[:, kt, n0:n1], start=(kt == 0), stop=(kt == NC - 1))
                y_sb = py.tile([128, 512], F32, tag="y_sb")
                nc.vector.tensor_tensor(y_sb[:, 0:n1 - n0], ps_y[:, 0:n1 - n0],
                                        bb_sb[:, C + n0:C + n1], OP.add)
                nc.sync.dma_start(io["y"][t * 128:(t + 1) * 128, n0:n1],
                                  y_sb[:, 0:n1 - n0])
            return emit

        # ---------------- fused phase 1 + 2 ----------------
        for u in make_qkv_units(0):
            run_unit(u)
        s_list = [make_s_units(h) for h in range(H)]
        o_u = {}
        for ft in range(NF):
            # interleave the pair's strips so both heads' exp streams start
            # immediately (ACT is the binding engine within attention)
            if ft < NF - 1:
                s_units = _weave(s_list[2 * ft], s_list[2 * ft + 1])
                if ft == NF - 2:
                    # head-start the last pair's exp: its weave is ACT-bound
                    s_units = s_units + s_list[2 * NF - 2][0:4]
            else:
                s_units = _weave(s_list[2 * NF - 2][4:], s_list[2 * NF - 1])
            qkv_units = make_qkv_units(ft + 1) if ft + 1 < NF else []
            if ft > 0:
                he, ho = 2 * ft - 2, 2 * ft - 1
                o_units = [o_u[he][0], o_u[ho][0], o_u[he][1], o_u[ho][1]]
            else:
                o_units = []
            if ft + 1 == NF:
                # last pair's first O chunks only need exp of strips 0..3;
                # weave them in so phase 3's early t-tiles are unblocked
                o_u[2 * ft] = make_o_units(2 * ft)
                o_u[2 * ft + 1] = make_o_units(2 * ft + 1)
                o_units += [o_u[2 * ft][0], o_u[2 * ft + 1][0]]
            for u in _weave(s_units, qkv_units, o_units):
                run_unit(u)
            if ft + 1 < NF:
                o_u[2 * ft] = make_o_units(2 * ft)
                o_u[2 * ft + 1] = make_o_units(2 * ft + 1)
        # epilogue: even head first (its exp finished earlier), odd head's
        # chunk last so its longer normalize chain overlaps early phase 3
        run_unit(o_u[2 * NF - 2][1])
        run_unit(o_u[2 * NF - 1][1])

        # ---------------- phase 3: projection ----------------
        for t in range(NT):
            for (n0, n1) in _chunks_512(0, C):
                run_unit(make_p3_unit(t, n0, n1))
        for item in pending:
            item[1]()
        pending.clear()


def declare_io(nc):
    return {
        "xT": nc.dram_tensor("xT", [C, T], F16, kind="ExternalInput").ap(),
        "wqkv": nc.dram_tensor("wqkv", [C, 3 * C], F16,
                               kind="ExternalInput").ap(),
        "bqkvT": nc.dram_tensor("bqkvT", [128, 3 * C // 128], F32,
                                kind="ExternalInput").ap(),
        "bias_bcast": nc.dram_tensor("bias_bcast", [128, 2 * C], F32,
                                     kind="ExternalInput").ap(),
        "wp": nc.dram_tensor("wp", [C, C], F16, kind="ExternalInput").ap(),
        "mask01": nc.dram_tensor("mask01", [128, 128], F16,
                                 kind="ExternalInput").ap(),
        "y": nc.dram_tensor("y", [T, C], F32, kind="ExternalOutput").ap(),
    }


def build_nc(repeat=1):
    from concourse import bacc
    import concourse.tile as tile
    nc = bacc.Bacc("TRN2", target_bir_lowering=False, debug=False,
                   enable_asserts=True, num_devices=N_CORES)
    io = declare_io(nc)
    with tile.TileContext(nc) as tc:
        for _ in range(repeat):
            _emit_attention(tc, io)
        for p in reversed(list(getattr(tc, "_v2_pools", {}).values())):
            p.release()
    nc.compile()
    return nc


def make_in_maps(x, c_attn_kernel, c_attn_bias, c_proj_kernel, c_proj_bias):
    # shuffle wqkv columns ft-major: [c, {q,k,v} x 768] -> [c, ft, {q,k,v}, 128]
    wqkv = np.ascontiguousarray(
        np.asarray(c_attn_kernel, dtype=np.float16)
        .reshape(C, 3, NF, 128).transpose(0, 2, 1, 3).reshape(C, 3 * C))
    bqkv = np.asarray(c_attn_bias, dtype=np.float32)
    bqkvT = np.ascontiguousarray(bqkv.reshape(3 * C // 128, 128).T)
    wp = np.ascontiguousarray(c_proj_kernel, dtype=np.float16)
    bp = np.asarray(c_proj_bias, dtype=np.float32)
    bias_bcast = np.ascontiguousarray(
        np.tile(np.concatenate([bqkv[2 * C:], bp]), (128, 1)))
    mask01 = np.triu(np.ones((128, 128), dtype=np.float16))
    in_maps = []
    for bb in range(N_CORES):
        m = {"xT": np.ascontiguousarray(np.asarray(x[bb]).T, dtype=np.float16),
             "wqkv": wqkv, "bqkvT": bqkvT, "bias_bcast": bias_bcast,
             "wp": wp, "mask01": mask01}
        in_maps.append(m)
    return in_maps


_NC_CACHE = None


def _get_nc():
    global _NC_CACHE
    if _NC_CACHE is None:
        _NC_CACHE = build_nc()
    return _NC_CACHE


def kernel(x, c_attn_kernel, c_attn_bias, c_proj_kernel, c_proj_bias):
    from concourse.bass_utils import run_bass_kernel_spmd
    x = np.asarray(x)
    assert x.shape == (B, T, C), x.shape
    nc = _get_nc()
    in_maps = make_in_maps(x, c_attn_kernel, c_attn_bias, c_proj_kernel,
                           c_proj_bias)
    res = run_bass_kernel_spmd(nc, in_maps, core_ids=list(range(N_CORES)))
    y = np.stack([res.results[bb]["y"]
                  for bb in range(N_CORES)]).astype(np.float32)
    return y
